# revision 28
# baseline (speedup 1.0000x reference)
"""2-layer GAT on 8 Trainium2 NeuronCores (Bass/Tile).

Strategy (edge-parallel, per the sharding hint: "each device holds a shard
of edge_index/edge_weight plus replicated (or gathered halo) node features"):

- Destination nodes are assigned round-robin by degree rank to the 8 cores;
  per core, dst slots are grouped 128 per partition-group. A group's edges
  live in a [128 partitions = dst slot, NCH columns] grid (self-loop first,
  padded slots masked to zero).
- The host stages each core's edge shard WITH ITS GATHERED HALO: for every
  grid slot, the raw source-node feature vector (bf16) — i.e. x[src] laid
  out in grid order. No indirect/gather DMA is ever issued on device (the
  SWDGE gather path costs ~16us/instruction on this runtime); the device
  streams the halo grid sequentially at full HBM bandwidth.
- ALL math runs on device: L1 projection of every (edge-slot) source vector
  on the PE (x@[W1|W1@A1] per grid column), segment softmax + weighted
  aggregation on DVE/ACT, ELU, L2 projection on PE.
- Between the two layer NEFFs the host re-shards the layer-1 activations
  the same way (gathered halo per edge slot) — replacing the cross-device
  exchange; segment reductions stay core-local because edges are
  dst-partitioned.
"""
import numpy as np
import ml_dtypes

N = 100000
E = 1600000
IN = 128
HID = 8
HEADS = 8
OUTC = 40
SLOPE = 0.2
NCORES = 8
P = 128
PCOLS = 6        # grid columns projected per PSUM tile (6*80 = 480 <= 512)

BF16 = ml_dtypes.bfloat16

_CACHE = {}


# --------------------------------------------------------------------------
# host-side index preprocessing (pure integer/layout work)
# --------------------------------------------------------------------------

def _host_prep(edge_index, edge_weight):
    src = np.asarray(edge_index[0], dtype=np.int64)
    dst = np.asarray(edge_index[1], dtype=np.int64)
    ew = np.asarray(edge_weight, dtype=np.float32)
    n = N

    deg = np.bincount(dst, minlength=n).astype(np.int64)          # real in-degree
    order = np.argsort(-deg, kind="stable")                        # degree desc
    core_of = np.empty(n, dtype=np.int64)
    slot_of = np.empty(n, dtype=np.int64)
    core_of[order] = np.arange(n) % NCORES
    slot_of[order] = np.arange(n) // NCORES                        # rank within core

    n_per_core = [int((core_of == k).sum()) for k in range(NCORES)]
    nmax = max(n_per_core)
    G = (nmax + P - 1) // P
    NV = G * P                                                     # slots per core

    # vperm[k][v] = real node at slot v of core k  (-1 = pad)
    vperm = np.full((NCORES, NV), -1, dtype=np.int64)
    vperm[core_of, slot_of] = np.arange(n)

    rowof = core_of * NV + slot_of                                 # [n] T2 row

    # group schedule: NCH per group = max (deg+1) among its 128 nodes, over cores
    degv = np.zeros((NCORES, NV), dtype=np.int64)
    degv[core_of, slot_of] = deg + 1                               # incl self-loop
    nch_per = degv.reshape(NCORES, G, P).max(axis=2)               # [NCORES, G]
    NCHS = nch_per.max(axis=0).astype(np.int64)                    # [G]
    NCHS = np.maximum(NCHS, 1)
    CSTART = np.concatenate([[0], np.cumsum(NCHS)])                # [G+1]
    TOT = int(CSTART[-1])

    # place edges: edge e of dst d -> (core_of[d], partition slot%P,
    # group slot//P, column 1 + rank-within-node); self-loop at column 0.
    e_core = core_of[dst]
    e_slot = slot_of[dst]
    ordr = np.argsort(e_core * NV + e_slot, kind="stable")
    sc, ss = e_core[ordr], e_slot[ordr]
    key = sc * NV + ss
    first = np.r_[True, key[1:] != key[:-1]]
    start_pos = np.flatnonzero(first)
    runlen = np.arange(len(key)) - np.repeat(start_pos, np.diff(np.r_[start_pos, len(key)]))
    e_col = np.empty(E, dtype=np.int64)
    e_col[ordr] = runlen + 1                                       # col 0 = self loop

    # per-core grid metadata
    SRCN = np.zeros((NCORES, TOT, P), dtype=np.int64)              # source node id
    M2 = np.zeros((NCORES, P, TOT), dtype=np.float32)              # ew * validity
    M1 = np.zeros((NCORES, P, TOT), dtype=np.float32)              # validity

    e_g = e_slot // P
    e_p = e_slot % P
    e_c = CSTART[e_g] + e_col
    SRCN[e_core, e_c, e_p] = src
    M2[e_core, e_p, e_c] = ew
    M1[e_core, e_p, e_c] = 1.0

    # self loops (only for real nodes)
    rn = np.arange(n)
    s_c = CSTART[slot_of // P]
    SRCN[core_of, s_c, slot_of % P] = rn
    M2[core_of, slot_of % P, s_c] = 1.0
    M1[core_of, slot_of % P, s_c] = 1.0

    # L2 grid source rows: rowof[SRCN]; pads point at row 0
    SRCROW = rowof[SRCN]                                           # [NCORES, TOT, P]

    return dict(G=G, NV=NV, NVG=NCORES * NV, NCHS=NCHS.tolist(),
                CSTART=CSTART.tolist(), TOT=TOT,
                m1_same=bool(np.array_equal(M1, M2)),
                vperm=vperm, SRCN=SRCN, SRCROW=SRCROW, M1=M1, M2=M2)


def _blockdiag(att):
    """att [H, C] -> [H*C, H] block diagonal."""
    h, c = att.shape
    A = np.zeros((h * c, h), dtype=np.float32)
    for i in range(h):
        A[i * c:(i + 1) * c, i] = att[i]
    return A


# --------------------------------------------------------------------------
# device program builders
# --------------------------------------------------------------------------

def _build_neff1(meta):
    import concourse.bacc as bacc
    import concourse.mybir as mybir
    import concourse.tile as tile
    import concourse.bass as bass
    from concourse.masks import make_identity

    G, NV, TOT = meta["G"], meta["NV"], meta["TOT"]
    NCHS, CSTART = meta["NCHS"], meta["CSTART"]
    m1_same = meta["m1_same"]
    bf16, f32 = mybir.dt.bfloat16, mybir.dt.float32
    AP = bass.AP
    H = HEADS

    TOTP = TOT * P
    XW = TOTP + NV + TOT
    nc = bacc.Bacc(num_devices=NCORES)
    # packed bf16 input: [ xgrid (TOT*P) | xnode (NV) | m2 mask (TOT) ]
    t_xgn = nc.dram_tensor("XGN", [IN, XW], bf16, kind="ExternalInput")
    # packed f32 weights [128, 187]:
    #   [:,0:64]=W1  [0:64,64:80]=A1  [0:64,80:120]=W2  [0:40,120:122]=A2
    #   [0,122:186]=b1
    t_WB = nc.dram_tensor("WB", [IN, 187], f32, kind="ExternalInput")
    if not m1_same:
        t_M1 = nc.dram_tensor("M1", [P, TOT], f32, kind="ExternalInput")

    t_AG = nc.dram_tensor("AGIN", [NV, 42], f32, kind="ExternalOutput")

    with tile.TileContext(nc) as tc:
        with tc.tile_pool(name="const", bufs=1) as cpool:

            ident = cpool.tile([P, P], f32)
            make_identity(nc, ident[:])

            w1eeb = cpool.tile([IN, 80], bf16)
            w1adb = cpool.tile([IN, H], bf16)
            w2e = cpool.tile([HEADS * HID, 42], f32)
            wb = cpool.tile([IN, 187], f32)
            nc.sync.dma_start(wb[:], t_WB[:])
            with tc.tile_pool(name="psetup", bufs=1, space="PSUM") as spp:
                # ---- build [W1 | W1@A1src] bf16 and W1@A1dst bf16 ----
                w1 = wb[:, 0:64]
                a1 = wb[0:64, 64:80]
                ps_w1t = spp.tile([HEADS * HID, IN], f32, space="PSUM")
                nc.tensor.transpose(out=ps_w1t[:], in_=w1, identity=ident[:])
                w1t = cpool.tile([HEADS * HID, IN], f32)
                nc.vector.tensor_copy(w1t[:], ps_w1t[:])
                ps_w1a = spp.tile([IN, 2 * HEADS], f32, space="PSUM")
                nc.tensor.matmul(out=ps_w1a[:], lhsT=w1t[:], rhs=a1,
                                 start=True, stop=True)
                nc.vector.tensor_copy(w1eeb[:, 0:64], w1)
                nc.vector.tensor_copy(w1eeb[:, 64:72], ps_w1a[:, 0:H])
                nc.vector.tensor_copy(w1eeb[:, 72:80], ps_w1a[:, H:2 * H])
                nc.vector.tensor_copy(w1adb[:], ps_w1a[:, H:2 * H])

                # ---- build W2e [64, 42] f32 = [W2 | W2@A2] ----
                w2 = wb[0:64, 80:120]
                a2 = wb[0:40, 120:122]
                ps_w2t = spp.tile([OUTC, HEADS * HID], f32, space="PSUM")
                nc.tensor.transpose(out=ps_w2t[:], in_=w2,
                                    identity=ident[0:HEADS * HID, 0:HEADS * HID])
                w2t = cpool.tile([OUTC, HEADS * HID], f32)
                nc.vector.tensor_copy(w2t[:], ps_w2t[:])
                ps_w2a = spp.tile([HEADS * HID, 2], f32, space="PSUM")
                nc.tensor.matmul(out=ps_w2a[:], lhsT=w2t[:], rhs=a2,
                                 start=True, stop=True)
                nc.vector.tensor_copy(w2e[:, 0:OUTC], w2)
                nc.vector.tensor_copy(w2e[:, OUTC:42], ps_w2a[:])

            # ---- bias1 replicated [128, 64] (row 0, cols 122:186 of WB) ----
            b1mat = cpool.tile([P, HEADS * HID], f32)
            nc.sync.dma_start(b1mat[:], AP(t_WB, 122, [[0, P], [1, HEADS * HID]]))

            # ---- masks resident in SBUF ----
            if not m1_same:
                m_m1 = cpool.tile([P, TOT], f32)
                nc.sync.dma_start(m_m1[:], t_M1[:])
            m2b = cpool.tile([P, TOT], bf16)
            nc.sync.dma_start(m2b[:], t_xgn[:, TOTP + NV:TOTP + NV + TOT])
            m_m2 = cpool.tile([P, TOT], f32)
            nc.vector.tensor_copy(m_m2[:], m2b[:])

            from contextlib import ExitStack
            _ps = ExitStack()
            pool = _ps.enter_context(tc.tile_pool(name="work", bufs=4))

            # ---- per-slot own adst: project local node features ----
            adall = cpool.tile([P, G, H], f32)
            adall_flat = adall[:].rearrange("p a b -> p (a b)")
            ADB = 4
            with tc.tile_pool(name="padp", bufs=2, space="PSUM") as app:
                for g0 in range(0, G, ADB):
                    gw = min(ADB, G - g0)
                    xn = pool.tile([IN, gw * P], bf16, tag="xn")
                    nc.sync.dma_start(xn[:],
                                      t_xgn[:, TOTP + g0 * P:TOTP + (g0 + gw) * P])
                    ps_ad = app.tile([P, ADB * H], f32, space="PSUM", tag="pad")
                    for j in range(gw):
                        nc.tensor.matmul(out=ps_ad[:, j * H:(j + 1) * H],
                                         lhsT=xn[:, j * P:(j + 1) * P], rhs=w1adb[:],
                                         start=True, stop=True)
                    nc.scalar.copy(adall_flat[:, g0 * H:(g0 + gw) * H],
                                   ps_ad[:, 0:gw * H])
            ppool = _ps.enter_context(tc.tile_pool(name="psum", bufs=2, space="PSUM"))
            pjpool = _ps.enter_context(tc.tile_pool(name="psumj", bufs=3, space="PSUM"))
            ado = adall[:].offset
            adp = adall[:].ap[0][0]
            m2o = m_m2[:].offset
            m2p = m_m2[:].ap[0][0]

            # ---- edge phase: supergroups of equal-NCH groups ----
            for (g0, S, NCH) in _sg_runs(NCHS, max_cols=48):
                SN = S * NCH
                cs = CSTART[g0]
                xg = pool.tile([IN, SN * P], bf16, tag="xg")
                nc.sync.dma_start(xg[:], t_xgn[:, cs * P:(cs + SN) * P])

                # project halo grid: gb[p, (s,c), 80] = [h | asrc | adst]
                gb = pool.tile([P, SN, 80], bf16, tag="gb")
                gb_flat = gb[:].rearrange("p a b -> p (a b)")
                for c0 in range(0, SN, PCOLS):
                    cw = min(PCOLS, SN - c0)
                    ps = pjpool.tile([P, PCOLS * 80], f32, space="PSUM", tag="pj")
                    for j in range(cw):
                        nc.tensor.matmul(
                            out=ps[:, j * 80:(j + 1) * 80],
                            lhsT=xg[:, (c0 + j) * P:(c0 + j + 1) * P],
                            rhs=w1eeb[:], start=True, stop=True)
                    nc.scalar.copy(gb_flat[:, c0 * 80:(c0 + cw) * 80],
                                   ps[:, 0:cw * 80])

                gbo = gb[:].offset
                pstep = gb[:].ap[0][0]
                # l[p,h,s,c] = asrc(by src) + adst(own dst)
                asrc_v = AP(gb.tensor, gbo + 64,
                            [[pstep, P], [1, H], [80 * NCH, S], [80, NCH]])
                ad_v = AP(adall.tensor, ado + g0 * H,
                          [[adp, P], [1, H], [H, S], [0, NCH]])
                lg = pool.tile([P, H, SN], f32, tag="lg")
                lgo = lg[:].offset
                lgp = lg[:].ap[0][0]
                lg_v = AP(lg.tensor, lgo, [[lgp, P], [SN, H], [NCH, S], [1, NCH]])
                nc.vector.tensor_tensor(lg_v, asrc_v, ad_v, mybir.AluOpType.add)
                lg_f = AP(lg.tensor, lgo, [[lgp, P], [1, H * SN]])
                lr = pool.tile([P, H * SN], f32, tag="lr")
                nc.vector.scalar_tensor_tensor(
                    out=lr[:], in0=lg_f, scalar=SLOPE, in1=lg_f,
                    op0=mybir.AluOpType.mult, op1=mybir.AluOpType.max)
                ex = pool.tile([P, H * SN], f32, tag="ex")
                nc.scalar.activation(ex[:], lr[:], mybir.ActivationFunctionType.Exp)
                exo = ex[:].offset
                exp_ = ex[:].ap[0][0]

                m2_v = AP(m_m2.tensor, m2o + cs,
                          [[m2p, P], [0, H], [NCH, S], [1, NCH]])
                ex_v = AP(ex.tensor, exo, [[exp_, P], [SN, H], [NCH, S], [1, NCH]])
                exm = pool.tile([P, H, SN], bf16, tag="exm")
                exmo = exm[:].offset
                exmp = exm[:].ap[0][0]
                exm_4 = AP(exm.tensor, exmo, [[exmp, P], [SN, H], [NCH, S], [1, NCH]])
                nc.vector.tensor_tensor(exm_4, ex_v, m2_v, mybir.AluOpType.mult)
                if m1_same:
                    exd_4 = exm_4
                else:
                    m1_v = AP(m_m1.tensor, m_m1[:].offset + cs,
                              [[m_m1[:].ap[0][0], P], [0, H], [NCH, S], [1, NCH]])
                    exd = pool.tile([P, H, SN], bf16, tag="exd")
                    exd_4 = AP(exd.tensor, exd[:].offset,
                               [[exd[:].ap[0][0], P], [SN, H], [NCH, S], [1, NCH]])
                    nc.vector.tensor_tensor(exd_4, ex_v, m1_v, mybir.AluOpType.mult)

                den = pool.tile([P, H, S], f32, tag="den")
                nc.vector.tensor_reduce(den[:], exd_4, mybir.AxisListType.X,
                                        mybir.AluOpType.add)
                den2 = pool.tile([P, H * S], f32, tag="den2")
                nc.vector.tensor_scalar_add(
                    den2[:], den[:].rearrange("p a b -> p (a b)"), 1e-16)
                rd = pool.tile([P, H, S], f32, tag="rd")
                nc.vector.reciprocal(rd[:].rearrange("p a b -> p (a b)"), den2[:])
                rdo = rd[:].offset
                rdp = rd[:].ap[0][0]

                # msgw[p, h, j, (s,c)] = h_src[p, (s,c), 8h+j] * exm[p, h, (s,c)]
                gh_v = AP(gb.tensor, gbo, [[pstep, P], [8, H], [1, HID], [80, SN]])
                exm_b = AP(exm.tensor, exmo, [[exmp, P], [SN, H], [0, HID], [1, SN]])
                msgw = pool.tile([P, H, HID, SN], bf16, tag="msgw")
                nc.vector.tensor_tensor(msgw[:], gh_v, exm_b, mybir.AluOpType.mult)
                mo = msgw[:].offset
                mp = msgw[:].ap[0][0]

                for s in range(S):
                    g = g0 + s
                    msg_s = AP(msgw.tensor, mo + s * NCH,
                               [[mp, P], [HID * SN, H], [SN, HID], [1, NCH]])
                    U = pool.tile([P, H, HID], f32, tag="U")
                    nc.vector.tensor_reduce(U[:], msg_s, mybir.AxisListType.X,
                                            mybir.AluOpType.add)

                    rd_v = AP(rd.tensor, rdo + s, [[rdp, P], [S, H], [0, HID]])
                    t2 = pool.tile([P, H * HID], f32, tag="t2")
                    nc.vector.tensor_tensor(
                        t2[:].rearrange("p (a b) -> p a b", a=H), U[:], rd_v,
                        mybir.AluOpType.mult)
                    t3 = pool.tile([P, H * HID], f32, tag="t3")
                    nc.vector.tensor_add(t3[:], t2[:], b1mat[:])
                    # elu
                    neg = pool.tile([P, H * HID], f32, tag="neg")
                    nc.vector.tensor_scalar_min(neg[:], t3[:], 0.0)
                    een = pool.tile([P, H * HID], f32, tag="een")
                    nc.scalar.activation(een[:], neg[:],
                                         mybir.ActivationFunctionType.Exp)
                    pos = pool.tile([P, H * HID], f32, tag="pos")
                    nc.vector.tensor_scalar_max(pos[:], t3[:], 0.0)
                    h1 = pool.tile([P, H * HID], f32, tag="h1")
                    nc.vector.scalar_tensor_tensor(
                        out=h1[:], in0=een[:], scalar=-1.0, in1=pos[:],
                        op0=mybir.AluOpType.add, op1=mybir.AluOpType.add)
                    # transpose + L2 projection: AGIN rows = [h2'|asrc2|adst2]
                    ps_tr = ppool.tile([H * HID, P], f32, space="PSUM", tag="ptr")
                    nc.tensor.transpose(out=ps_tr[:], in_=h1[:], identity=ident[:])
                    o1 = pool.tile([H * HID, P], f32, tag="o1")
                    nc.vector.tensor_copy(o1[:], ps_tr[:])
                    ps2 = ppool.tile([P, 42], f32, space="PSUM", tag="p2")
                    nc.tensor.matmul(out=ps2[:], lhsT=o1[:], rhs=w2e[:],
                                     start=True, stop=True)
                    og = pool.tile([P, 42], f32, tag="og")
                    nc.scalar.copy(og[:], ps2[:])
                    nc.sync.dma_start(t_AG[g * P:(g + 1) * P, :], og[:])

            _ps.close()

    nc.finalize()
    return nc


def _sg_runs(NCHS, max_cols=64, max_s=8):
    """Merge consecutive equal-NCH groups into supergroup runs."""
    runs = []
    g, G = 0, len(NCHS)
    while g < G:
        nch = NCHS[g]
        s = 1
        while (g + s < G and NCHS[g + s] == nch
               and (s + 1) * nch <= max_cols and s + 1 <= max_s):
            s += 1
        runs.append((g, s, nch))
        g += s
    return runs


def _build_neff2(meta):
    import concourse.bacc as bacc
    import concourse.mybir as mybir
    import concourse.tile as tile
    import concourse.bass as bass

    G, NV, TOT = meta["G"], meta["NV"], meta["TOT"]
    NCHS, CSTART = meta["NCHS"], meta["CSTART"]
    m1_same = meta["m1_same"]
    bf16, f32 = mybir.dt.bfloat16, mybir.dt.float32
    AP = bass.AP

    C2 = TOT * 42
    nc = bacc.Bacc(num_devices=NCORES)
    # packed bf16 input: [ halo grid (TOT*42) | ad2 (G) | b2 (40) | m2 (TOT) ]
    t_g2 = nc.dram_tensor("G2E", [P, C2 + G + OUTC + TOT], bf16,
                          kind="ExternalInput")
    if not m1_same:
        t_M1 = nc.dram_tensor("M1", [P, TOT], f32, kind="ExternalInput")
    t_OUT = nc.dram_tensor("OUT2", [NV, OUTC], f32, kind="ExternalOutput")

    with tile.TileContext(nc) as tc:
        with tc.tile_pool(name="const", bufs=1) as cpool, \
             tc.tile_pool(name="work", bufs=6) as pool:

            b2b = cpool.tile([P, OUTC], bf16)
            nc.sync.dma_start(b2b[:], t_g2[:, C2 + G:C2 + G + OUTC])
            b2mat = cpool.tile([P, OUTC], f32)
            nc.vector.tensor_copy(b2mat[:], b2b[:])
            if not m1_same:
                m_m1 = cpool.tile([P, TOT], f32)
                nc.sync.dma_start(m_m1[:], t_M1[:])
            m2b = cpool.tile([P, TOT], bf16)
            nc.sync.dma_start(m2b[:], t_g2[:, C2 + G + OUTC:C2 + G + OUTC + TOT])
            m_m2 = cpool.tile([P, TOT], f32)
            nc.vector.tensor_copy(m_m2[:], m2b[:])
            ad2 = cpool.tile([P, G], bf16)
            nc.sync.dma_start(ad2[:], t_g2[:, C2:C2 + G])
            ad2o = ad2[:].offset
            ad2p = ad2[:].ap[0][0]
            m2o = m_m2[:].offset
            m2p = m_m2[:].ap[0][0]

            for (g0, S, NCH) in _sg_runs(NCHS):
                SN = S * NCH
                cs = CSTART[g0]
                # halo grid slice [128, S*NCH, 42] bf16: [h2'(40)|asrc2|adst2]
                gb = pool.tile([P, SN, 42], bf16, tag="gb")
                nc.sync.dma_start(gb[:].rearrange("p a b -> p (a b)"),
                                  t_g2[:, cs * 42:(cs + SN) * 42])
                gbo = gb[:].offset
                pstep = gb[:].ap[0][0]

                # logits: asrc(by src) + adst(own dst), [P, S, NCH]
                asrc_v = AP(gb.tensor, gbo + 40,
                            [[pstep, P], [42 * NCH, S], [42, NCH]])
                ad_v = AP(ad2.tensor, ad2o + g0, [[ad2p, P], [1, S], [0, NCH]])
                lg = pool.tile([P, S, NCH], f32, tag="lg")
                nc.vector.tensor_tensor(lg[:], asrc_v, ad_v, mybir.AluOpType.add)
                lr = pool.tile([P, SN], f32, tag="lr")
                nc.vector.scalar_tensor_tensor(
                    out=lr[:], in0=lg[:].rearrange("p a b -> p (a b)"),
                    scalar=SLOPE, in1=lg[:].rearrange("p a b -> p (a b)"),
                    op0=mybir.AluOpType.mult, op1=mybir.AluOpType.max)
                ex = pool.tile([P, SN], f32, tag="ex")
                nc.scalar.activation(ex[:], lr[:], mybir.ActivationFunctionType.Exp)

                m2_v = AP(m_m2.tensor, m2o + cs, [[m2p, P], [1, SN]])
                exm = pool.tile([P, SN], bf16, tag="exm")
                nc.vector.tensor_tensor(exm[:], ex[:], m2_v, mybir.AluOpType.mult)
                if m1_same:
                    exd = exm[:]
                else:
                    m1_v = AP(m_m1.tensor, m_m1[:].offset + cs, [[m_m1[:].ap[0][0], P], [1, SN]])
                    exd_t = pool.tile([P, SN], bf16, tag="exd")
                    nc.vector.tensor_tensor(exd_t[:], ex[:], m1_v, mybir.AluOpType.mult)
                    exd = exd_t[:]

                den = pool.tile([P, S], f32, tag="den")
                nc.vector.tensor_reduce(
                    den[:],
                    AP(exd.tensor, exd.offset, [[exd.ap[0][0], P], [NCH, S], [1, NCH]]),
                    mybir.AxisListType.X, mybir.AluOpType.add)
                den2 = pool.tile([P, S], f32, tag="den2")
                nc.vector.tensor_scalar_add(den2[:], den[:], 1e-16)
                rd = pool.tile([P, S], f32, tag="rd")
                nc.vector.reciprocal(rd[:], den2[:])

                # msgw[p, o, (s,c)] = h2[p, (s,c), o] * exm[p, (s,c)]
                gh_v = AP(gb.tensor, gbo, [[pstep, P], [1, OUTC], [42, SN]])
                exm_v = AP(exm.tensor, exm[:].offset,
                           [[exm[:].ap[0][0], P], [0, OUTC], [1, SN]])
                msgw = pool.tile([P, OUTC, SN], bf16, tag="msgw")
                nc.vector.tensor_tensor(msgw[:], gh_v, exm_v, mybir.AluOpType.mult)
                # U[p, o, s] = sum_c msgw — one 4-dim reduce for the whole run
                mo = msgw[:].offset
                mp = msgw[:].ap[0][0]
                U = pool.tile([P, OUTC, S], f32, tag="U")
                nc.vector.tensor_reduce(
                    U[:], AP(msgw.tensor, mo, [[mp, P], [SN, OUTC], [NCH, S], [1, NCH]]),
                    mybir.AxisListType.X, mybir.AluOpType.add)

                # out[p, s, o] = U * rd + b2  (tiles laid [P, S, OUTC] for DMA)
                t2 = pool.tile([P, S, OUTC], f32, tag="t2")
                t2o = t2[:].offset
                t2p = t2[:].ap[0][0]
                t2_osv = AP(t2.tensor, t2o, [[t2p, P], [1, OUTC], [OUTC, S]])
                rd_v = AP(rd.tensor, rd[:].offset,
                          [[rd[:].ap[0][0], P], [0, OUTC], [1, S]])
                nc.vector.tensor_tensor(t2_osv, U[:], rd_v, mybir.AluOpType.mult)
                t3 = pool.tile([P, S, OUTC], f32, tag="t3")
                b2_v = AP(b2mat.tensor, b2mat[:].offset,
                          [[b2mat[:].ap[0][0], P], [0, S], [1, OUTC]])
                nc.vector.tensor_tensor(t3[:], t2[:], b2_v, mybir.AluOpType.add)
                # rows (g0+s)*128 + p of OUT2
                dst = AP(t_OUT, g0 * P * OUTC,
                         [[OUTC, P], [P * OUTC, S], [1, OUTC]])
                nc.sync.dma_start(dst, t3[:])

    nc.finalize()
    return nc


# --------------------------------------------------------------------------
# host staging (pure layout/sharding, no FLOPs on tensor data)
# --------------------------------------------------------------------------

def _stage_xgrids(x, meta):
    """Per core, one packed bf16 tensor [IN, TOT*P + NV + TOT]:
    [ halo grid x[src] (feature-major) | local node features | m2 mask ]."""
    xb = np.ascontiguousarray(x).astype(BF16)
    TOT, NV = meta["TOT"], meta["NV"]
    out = []
    for k in range(NCORES):
        xgn = np.empty((IN, TOT * P + NV + TOT), dtype=BF16)
        rows = xb[meta["SRCN"][k].reshape(-1)]                # [TOT*P, IN]
        xgn[:, 0:TOT * P] = rows.T
        vp = meta["vperm"][k]
        xn = np.zeros((NV, IN), dtype=BF16)
        valid = vp >= 0
        xn[valid] = xb[vp[valid]]
        xgn[:, TOT * P:TOT * P + NV] = xn.T
        xgn[:, TOT * P + NV:] = meta["M2"][k].astype(BF16)
        out.append(xgn)
    return out


def _stage_grid2(T2full, bias2, meta):
    """Per core, one packed bf16 tensor [P, TOT*42 + G + OUTC + TOT]:
    [ L2 halo grid (T2 row per slot, partition-major) | own adst2 | b2 | m2 ]."""
    out = []
    NV, G, TOT = meta["NV"], meta["G"], meta["TOT"]
    T2b = T2full.astype(BF16)
    b2b = np.broadcast_to(bias2.astype(BF16), (P, OUTC))
    for k in range(NCORES):
        g2e = np.empty((P, TOT * 42 + G + OUTC + TOT), dtype=BF16)
        rows = T2b[meta["SRCROW"][k].reshape(-1)]             # [TOT*P, 42]
        g2e[:, 0:TOT * 42] = rows.reshape(TOT, P, 42).transpose(1, 0, 2) \
                                 .reshape(P, TOT * 42)
        own = k * NV + np.arange(NV)
        g2e[:, TOT * 42:TOT * 42 + G] = T2b[own, 41].reshape(G, P).T
        g2e[:, TOT * 42 + G:TOT * 42 + G + OUTC] = b2b
        g2e[:, TOT * 42 + G + OUTC:] = meta["M2"][k].astype(BF16)
        out.append(g2e)
    return out


# --------------------------------------------------------------------------
# entry point
# --------------------------------------------------------------------------

def _run_with_retry(run, args, tries=3):
    """One transient 'mesh desynced' style failure shouldn't kill the call."""
    import time as _time
    for i in range(tries):
        try:
            return run.run(args)
        except Exception:
            if i == tries - 1:
                raise
            _time.sleep(3.0)


def kernel(x, edge_index, edge_weight, W1, att_src1, att_dst1, bias1,
           W2, att_src2, att_dst2, bias2):
    SpmdRunner = _inline_runner()

    x = np.asarray(x, dtype=np.float32)
    W1 = np.asarray(W1, dtype=np.float32)
    W2 = np.asarray(W2, dtype=np.float32)
    bias1 = np.asarray(bias1, dtype=np.float32)
    bias2 = np.asarray(bias2, dtype=np.float32)

    import hashlib
    h = hashlib.sha1()
    h.update(np.ascontiguousarray(edge_index).tobytes())
    h.update(np.ascontiguousarray(edge_weight).tobytes())
    key = h.hexdigest()
    if _CACHE.get("key") != key:
        _CACHE.clear()
        _CACHE["key"] = key
        _CACHE["meta"] = _host_prep(edge_index, edge_weight)
    meta = _CACHE["meta"]

    xgns = _stage_xgrids(x, meta)
    A1 = np.concatenate(
        [_blockdiag(np.asarray(att_src1, np.float32)),
         _blockdiag(np.asarray(att_dst1, np.float32))], axis=1)    # [64, 16]
    A2 = np.concatenate(
        [np.asarray(att_src2, np.float32).reshape(OUTC, 1),
         np.asarray(att_dst2, np.float32).reshape(OUTC, 1)], axis=1)  # [40, 2]
    WB = np.zeros((IN, 187), np.float32)
    WB[:, 0:64] = W1
    WB[0:64, 64:80] = A1
    WB[0:64, 80:120] = W2
    WB[0:40, 120:122] = A2
    WB[0, 122:186] = bias1

    if "nc1" not in _CACHE:
        _CACHE["nc1"] = _build_neff1(meta)
        _CACHE["run1"] = SpmdRunner(_CACHE["nc1"], NCORES)
    run1 = _CACHE["run1"]

    in_maps1 = []
    for k in range(NCORES):
        m = {"XGN": xgns[k], "WB": WB}
        if not meta["m1_same"]:
            m["M1"] = np.asarray(meta["M1"][k])
        in_maps1.append(m)
    args1 = run1.prepare(in_maps1)
    _CACHE["args1_cached"] = args1
    res1 = run1.results(_run_with_retry(run1, args1))

    T2full = np.concatenate([r["AGIN"] for r in res1], axis=0)     # [NVG, 42]

    g2es = _stage_grid2(T2full, bias2, meta)

    if "nc2" not in _CACHE:
        _CACHE["nc2"] = _build_neff2(meta)
        _CACHE["run2"] = SpmdRunner(_CACHE["nc2"], NCORES)
    run2 = _CACHE["run2"]

    in_maps2 = []
    for k in range(NCORES):
        m = {"G2E": g2es[k]}
        if not meta["m1_same"]:
            m["M1"] = np.asarray(meta["M1"][k])
        in_maps2.append(m)
    args2 = run2.prepare(in_maps2)
    _CACHE["args2_cached"] = args2
    res2 = run2.results(_run_with_retry(run2, args2))

    out = np.zeros((N, OUTC), dtype=np.float32)
    for k in range(NCORES):
        vp = meta["vperm"][k]
        valid = vp >= 0
        out[vp[valid]] = res2[k]["OUT2"][np.flatnonzero(valid)]
    return out


def _inline_runner():
    """Self-contained copy of runner.SpmdRunner for harness environments."""
    import time
    import jax
    from jax.sharding import Mesh, PartitionSpec
    from jax.experimental.shard_map import shard_map
    import concourse.mybir as mybir
    from concourse import bass2jax
    from concourse.bass2jax import _bass_exec_p, partition_id_tensor

    class SpmdRunner:
        def __init__(self, nc, n_cores):
            bass2jax.install_neuronx_cc_hook()
            self.nc = nc
            self.n_cores = n_cores
            in_names, out_names, out_avals, zero_outs = [], [], [], []
            partition_name = (nc.partition_id_tensor.name
                              if nc.partition_id_tensor else None)
            for alloc in nc.m.functions[0].allocations:
                if not isinstance(alloc, mybir.MemoryLocationSet):
                    continue
                name = alloc.memorylocations[0].name
                if alloc.kind == "ExternalInput":
                    if name != partition_name:
                        in_names.append(name)
                elif alloc.kind == "ExternalOutput":
                    shape = tuple(alloc.tensor_shape)
                    dtype = mybir.dt.np(alloc.dtype)
                    out_names.append(name)
                    out_avals.append(jax.core.ShapedArray(shape, dtype))
                    zero_outs.append(np.zeros(shape, dtype))
            self.in_names = list(in_names)
            self.out_names, self.out_avals, self.zero_outs = out_names, out_avals, zero_outs
            n_params, n_outs = len(in_names), len(out_avals)
            all_in = in_names + out_names + ([partition_name] if partition_name else [])

            def _body(*args):
                operands = list(args)
                if partition_name is not None:
                    operands.append(partition_id_tensor())
                return tuple(_bass_exec_p.bind(
                    *operands, out_avals=tuple(out_avals), in_names=tuple(all_in),
                    out_names=tuple(out_names), lowering_input_output_aliases=(),
                    sim_require_finite=True, sim_require_nnan=True, nc=nc))

            devices = jax.devices()[:n_cores]
            mesh = Mesh(np.asarray(devices), ("core",))
            in_specs = (PartitionSpec("core"),) * (n_params + n_outs)
            out_specs = (PartitionSpec("core"),) * n_outs
            self.fn = jax.jit(shard_map(_body, mesh=mesh, in_specs=in_specs,
                                        out_specs=out_specs, check_rep=False),
                              keep_unused=True)
            self.n_params, self.n_outs = n_params, n_outs
            self._mesh = mesh

        def prepare(self, in_maps, device_put=True):
            import jax
            from jax.sharding import PartitionSpec
            per_core = [[np.asarray(m[nm]) for nm in self.in_names] for m in in_maps]
            args = [np.concatenate([per_core[c][i] for c in range(self.n_cores)], axis=0)
                    for i in range(self.n_params)]
            args += [np.zeros((self.n_cores * z.shape[0], *z.shape[1:]), z.dtype)
                     for z in self.zero_outs]
            if device_put:
                sh = jax.sharding.NamedSharding(self._mesh, PartitionSpec("core"))
                args = [jax.device_put(a, sh) for a in args]
                jax.block_until_ready(args)
            return args

        def run(self, args):
            import jax
            outs = self.fn(*args)
            jax.block_until_ready(outs)
            return outs

        def results(self, outs):
            return [{nm: np.asarray(outs[i]).reshape(
                        self.n_cores, *self.out_avals[i].shape)[c]
                     for i, nm in enumerate(self.out_names)}
                    for c in range(self.n_cores)]

    return SpmdRunner
